# revision 46
# baseline (speedup 1.0000x reference)
"""Multi-head cross-batch attention (B=4096, d_model=512, H=8 heads) on 8 TRN2 cores.

Sharding: one head per NeuronCore (tensor-parallel over H). Each core computes
its head's Q/K/V projections from a replicated (pre-transposed) x, the full
[4096, 4096] score block for that head, softmax (transposed layout, denominator
via a ones-column in V), attn @ V, and its partial out-projection
Y_h = attn_h @ Wo[:, h*64:(h+1)*64].T. Host sums the 8 partials and adds bo.

Layout notes (per core):
  - xT [512, 4096] (c on partitions) is fed from host so every matmul can
    contract over the partition dim without any on-device transpose of x.
  - QT/KT are stored duplicated across partition halves ([128, 4096]) so score
    matmuls can be row-packed two-at-a-time into the 128x128 PE array (the
    contraction dim is only 64).
  - Scores are computed transposed (ST[j, m]) so softmax's sum over keys j can
    ride the attn@V matmul: V is augmented with a ones column, making the
    accumulated output row 64 equal to sum_j exp(s). No max-subtraction is
    needed: scores are O(1) here (verified), so exp cannot overflow.
  - Normalization commutes with the out-projection, so Y_un rows are scaled by
    1/r with a per-partition tensor_scalar after the final matmul. r (living in
    a free-dim row) is transposed to partitions with a K=1 matmul.
  - Matmul inputs are bf16 (1 PE pass vs 2 for fp32, fast weight load);
    accumulation is fp32 in PSUM, exp inputs and the softmax denominator stay
    fp32. Emulated error of this variant vs the f64 reference: l2 rel 1.9e-3.
  - Emission is software-pipelined: attnV trails its scores/exp by one group so
    the PE's in-order queue never blocks on an in-flight exp; each chunk's
    output phase is emitted mid-way through the next chunk; chunk 0's score
    groups are interleaved into the projection loop as x arrives n-major.
"""

import sys

if "/opt/trn_rl_repo" not in sys.path:
    sys.path.insert(0, "/opt/trn_rl_repo")

import ml_dtypes
import numpy as np

import concourse.bass as bass
import concourse.tile as tile
from concourse import bacc, mybir

B = 4096
D = 512
H = 8
DK = 64
MC = 512  # query-chunk (m) width
N_MC = B // MC  # 8
JB = B // 128  # 32 j-blocks of 128 keys
F32 = mybir.dt.float32
BF16 = mybir.dt.bfloat16
MM_DT = BF16
NP_MM_DT = ml_dtypes.bfloat16 if MM_DT == BF16 else np.float32

# j-blocks per score/exp group: 3 blocks = 1536 floats = 3 PSUM banks.
# PSUM budget: 2x3 (score staging) + 1 (attnV accum) + 1 (outproj/rT/Vproj) = 8.
JGROUPS = [(0, 3), (3, 3), (6, 3), (9, 3), (12, 3), (15, 3), (18, 3), (21, 3), (24, 3), (27, 3), (30, 2)]

_NC_CACHE = None


def build_nc():
    nc = bacc.Bacc()

    xt = nc.dram_tensor("xt", [D, B], MM_DT, kind="ExternalInput")
    wqt = nc.dram_tensor("wqt", [D, 128], MM_DT, kind="ExternalInput")  # [c, d dup'd]
    wkt = nc.dram_tensor("wkt", [D, 128], MM_DT, kind="ExternalInput")
    wvt = nc.dram_tensor("wvt", [D, DK], MM_DT, kind="ExternalInput")  # [c, d]
    bqd = nc.dram_tensor("bqd", [128, 1], F32, kind="ExternalInput")  # bias dup'd
    bkd = nc.dram_tensor("bkd", [128, 1], F32, kind="ExternalInput")
    bvr = nc.dram_tensor("bvr", [1, DK], MM_DT, kind="ExternalInput")  # bias as row
    wot = nc.dram_tensor("wot", [DK, D], MM_DT, kind="ExternalInput")
    y = nc.dram_tensor("y", [B, D], F32, kind="ExternalOutput")

    with tile.TileContext(nc) as tc:
        with (
            tc.tile_pool(name="const", bufs=1) as const,
            tc.tile_pool(name="epool", bufs=16) as epool,
            tc.tile_pool(name="otpool", bufs=2) as otpool,
            tc.tile_pool(name="ypool", bufs=3) as ypool,
            tc.tile_pool(name="rpool", bufs=4) as rpool,
            tc.tile_pool(name="score_ps", bufs=2, space="PSUM") as score_ps,
            tc.tile_pool(name="attnv_ps", bufs=1, space="PSUM") as attnv_ps,
            tc.tile_pool(name="out_ps", bufs=1, space="PSUM") as out_ps,
        ):
            # ---- persistent SBUF ----
            x_sb = const.tile([128, 4 * B], MM_DT)  # 4 c-chunks side by side
            wq_sb = const.tile([128, 512], MM_DT)  # 4 c-chunks of [128,128]
            wk_sb = const.tile([128, 512], MM_DT)
            wv_sb = const.tile([128, 4 * DK], MM_DT)  # 4 c-chunks of [128,64]
            bq_sb = const.tile([128, 1], F32)
            bk_sb = const.tile([128, 1], F32)
            bv_sb = const.tile([1, DK], MM_DT)
            wot_sb = const.tile([DK, D], MM_DT)
            ones_sb = const.tile([128, 1], F32)
            onesr_sb = const.tile([1, 128], MM_DT)
            qt_sb = const.tile([128, B], MM_DT)  # QT dup'd across partition halves
            kt_sb = const.tile([128, B], MM_DT)
            vp_sb = const.tile([128, JB * (DK + 1)], MM_DT)  # [V | 1] per j-block

            # ---- input DMAs ----
            # Weights first (tiny, gate every projection matmul), one descriptor
            # per tensor via 3D APs. x streams n-major as 8 consolidated DMAs
            # split across two otherwise-idle engines so projections/scores/exp
            # start after ~1/8 of the x DMA.
            nc.sync.dma_start(
                out=wq_sb[:].rearrange("p (c n) -> p c n", c=4),
                in_=wqt[:].rearrange("(c p) n -> p c n", p=128),
            )
            nc.sync.dma_start(
                out=wk_sb[:].rearrange("p (c n) -> p c n", c=4),
                in_=wkt[:].rearrange("(c p) n -> p c n", p=128),
            )
            nc.sync.dma_start(
                out=wv_sb[:].rearrange("p (c n) -> p c n", c=4),
                in_=wvt[:].rearrange("(c p) n -> p c n", p=128),
            )
            nc.sync.dma_start(out=bq_sb[:], in_=bqd[:])
            nc.sync.dma_start(out=bk_sb[:], in_=bkd[:])
            nc.sync.dma_start(out=bv_sb[:], in_=bvr[:])
            nc.sync.dma_start(out=wot_sb[:], in_=wot[:])
            x_sb3 = x_sb[:].rearrange("p (c n) -> p c n", c=4)
            xt3 = xt[:].rearrange("(c p) n -> p c n", p=128)
            x_dma_eng = [nc.gpsimd, nc.scalar]
            for n in range(N_MC):
                x_dma_eng[n % 2].dma_start(
                    out=x_sb3[:, :, n * MC : (n + 1) * MC],
                    in_=xt3[:, :, n * MC : (n + 1) * MC],
                )
            nc.vector.memset(ones_sb[:], 1.0)
            nc.vector.memset(onesr_sb[:], 1.0)
            nc.vector.memset(vp_sb[:], 1.0)  # ones columns; V data overwrites the rest

            # ---- emission helpers ----
            def emit_qk(n):
                # one 2-bank PSUM unit holds Q (cols 0:512) and K (cols 512:1024)
                pp = score_ps.tile([128, 2 * MC], F32, tag="score")
                for off, w_sb in ((0, wq_sb), (MC, wk_sb)):
                    for c in range(4):
                        nc.tensor.matmul(
                            pp[:, off : off + MC],
                            w_sb[:, c * 128 : (c + 1) * 128],
                            x_sb[:, c * B + n * MC : c * B + (n + 1) * MC],
                            start=(c == 0),
                            stop=(c == 3),
                        )
                for off, b_sb, dst in ((0, bq_sb, qt_sb), (MC, bk_sb, kt_sb)):
                    nc.vector.tensor_scalar(
                        out=dst[:, n * MC : (n + 1) * MC], in0=pp[:, off : off + MC],
                        scalar1=b_sb[:], scalar2=None, op0=mybir.AluOpType.add,
                    )

            def emit_v4(n):
                # V for j-blocks 4n..4n+3 in one 1-bank PSUM unit, one strided drain
                t0 = 4 * n
                vps = out_ps.tile([128, 4 * DK], F32, tag="out")
                for k in range(4):
                    dst = vps[:, k * DK : (k + 1) * DK]
                    for c in range(4):
                        nc.tensor.matmul(
                            dst,
                            x_sb[:, c * B + (t0 + k) * 128 : c * B + (t0 + k + 1) * 128],
                            wv_sb[:, c * DK : (c + 1) * DK],
                            start=(c == 0),
                            stop=False,
                        )
                    nc.tensor.matmul(dst, onesr_sb[:], bv_sb[:], start=False, stop=True)
                nc.vector.tensor_copy(
                    vp_sb[:].rearrange("p (t e) -> p t e", e=DK + 1)[:, t0 : t0 + 4, 0:DK],
                    vps[:].rearrange("p (t e) -> p t e", e=DK),
                )

            def emit_sc(mc, g0, gn):
                m0 = mc * MC
                sp = score_ps.tile([128, gn * MC], F32, tag="score")
                et = epool.tile([128, gn * MC], MM_DT, tag="E")
                for k in range(gn):
                    jb = g0 + k
                    h0 = 64 * (jb % 2)
                    nc.tensor.matmul(
                        sp[:, k * MC : (k + 1) * MC],
                        kt_sb[h0 : h0 + 64, jb * 128 : (jb + 1) * 128],
                        qt_sb[h0 : h0 + 64, m0 : m0 + MC],
                        start=True,
                        stop=True,
                    )
                nc.scalar.activation(et[:], sp[:], mybir.ActivationFunctionType.Exp, scale=0.125)
                return et

            def emit_av(mc, g0, gn, et, av):
                for k in range(gn):
                    jb = g0 + k
                    nc.tensor.matmul(
                        av[:],
                        vp_sb[:, jb * (DK + 1) : (jb + 1) * (DK + 1)],
                        et[:, k * MC : (k + 1) * MC],
                        start=(jb == 0),
                        stop=(jb == JB - 1),
                    )

            def emit_otcopy(av):
                # r row (f32, feeds the K=1 transpose matmul) + bf16 numerator
                ot_f = otpool.tile([DK + 1, MC], F32, tag="otf")
                nc.vector.tensor_copy(ot_f[DK : DK + 1, :], av[DK : DK + 1, :])
                ot_b = otpool.tile([DK, MC], MM_DT, tag="otb")
                nc.vector.tensor_copy(ot_b[:], av[0:DK, :])
                return ot_f, ot_b

            def emit_output(mc, ot_f, ot_b):
                m0 = mc * MC
                for q in range(4):
                    rt = out_ps.tile([128, MC], F32, tag="out")
                    nc.tensor.matmul(
                        rt[:, 0:1],
                        ot_f[DK : DK + 1, q * 128 : (q + 1) * 128],
                        ones_sb[DK : DK + 1, 0:1],
                        start=True,
                        stop=True,
                    )
                    rv = rpool.tile([128, 1], F32, tag="rinv")
                    nc.vector.reciprocal(rv[:], rt[:, 0:1])
                    yp = out_ps.tile([128, MC], F32, tag="out")
                    nc.tensor.matmul(yp[:], ot_b[:, q * 128 : (q + 1) * 128], wot_sb[:], start=True, stop=True)
                    ysb = ypool.tile([128, MC], F32, tag="y")
                    nc.vector.tensor_scalar(
                        out=ysb[:], in0=yp[:], scalar1=rv[:], scalar2=None, op0=mybir.AluOpType.mult
                    )
                    nc.sync.dma_start(out=y[m0 + q * 128 : m0 + (q + 1) * 128, :], in_=ysb[:])

            # ---- software-pipelined main emission ----
            # attnV for a group is emitted one group behind its scores/exp, so
            # the PE's in-order queue never blocks on an in-flight exp. Each
            # chunk's output phase is emitted mid-way through the next chunk.
            state = {"prev": None, "av": None, "out_pending": None}

            def drain_prev():
                if state["prev"] is None:
                    return
                mc, g0, gn, et = state["prev"]
                state["prev"] = None
                if state["av"] is None:
                    state["av"] = attnv_ps.tile([DK + 1, MC], F32, tag="attnv", name="av")
                emit_av(mc, g0, gn, et, state["av"])
                if g0 + gn == JB:  # chunk complete
                    ot_f, ot_b = emit_otcopy(state["av"])
                    state["av"] = None
                    if state["out_pending"] is not None:
                        emit_output(*state["out_pending"])
                    state["out_pending"] = (mc, ot_f, ot_b)
                elif g0 >= 12 and state["out_pending"] is not None and state["out_pending"][0] == mc - 1:
                    emit_output(*state["out_pending"])
                    state["out_pending"] = None

            def push(mc, g0, gn):
                et = emit_sc(mc, g0, gn)
                drain_prev()
                state["prev"] = (mc, g0, gn, et)

            # projections interleaved with chunk 0 (x arrives n-major)
            gi = 0
            for n in range(N_MC):
                emit_qk(n)
                emit_v4(n)
                # chunk-0 groups whose j-blocks (and V blocks) are now projected
                while gi < len(JGROUPS) and JGROUPS[gi][0] + JGROUPS[gi][1] <= 4 * n + 4:
                    push(0, *JGROUPS[gi])
                    gi += 1
            while gi < len(JGROUPS):
                push(0, *JGROUPS[gi])
                gi += 1
            for mc in range(1, N_MC):
                for g0, gn in JGROUPS:
                    push(mc, g0, gn)
            drain_prev()
            emit_output(*state["out_pending"])
    nc.finalize()
    return nc


def _get_nc():
    global _NC_CACHE
    if _NC_CACHE is None:
        _NC_CACHE = build_nc()
    return _NC_CACHE


def make_in_maps(x, Wq, bq, Wk, bk, Wv, bv, Wo, bo):
    xT = np.ascontiguousarray(np.asarray(x, dtype=np.float32).T).astype(NP_MM_DT)
    maps = []
    for h in range(H):
        s = slice(h * DK, (h + 1) * DK)
        wqT = np.asarray(Wq, np.float32)[s, :].T  # [512, 64]
        wkT = np.asarray(Wk, np.float32)[s, :].T
        maps.append(
            {
                "xt": xT,
                "wqt": np.ascontiguousarray(np.concatenate([wqT, wqT], axis=1)).astype(NP_MM_DT),
                "wkt": np.ascontiguousarray(np.concatenate([wkT, wkT], axis=1)).astype(NP_MM_DT),
                "wvt": np.ascontiguousarray(np.asarray(Wv, np.float32)[s, :].T).astype(NP_MM_DT),
                "bqd": np.ascontiguousarray(np.tile(np.asarray(bq, np.float32)[s], 2).reshape(128, 1)),
                "bkd": np.ascontiguousarray(np.tile(np.asarray(bk, np.float32)[s], 2).reshape(128, 1)),
                "bvr": np.ascontiguousarray(np.asarray(bv, np.float32)[s].reshape(1, DK)).astype(NP_MM_DT),
                "wot": np.ascontiguousarray(np.asarray(Wo, np.float32)[:, s].T).astype(NP_MM_DT),
            }
        )
    return maps


def _ensure_ntff_hook_shim():
    # The image's antenv package lacks axon_hooks; bass_utils imports it when
    # tracing is requested (including via the BASS_TRACE env var). Register a
    # ctypes-backed shim so that path works regardless of environment.
    if "antenv.axon_hooks" in sys.modules:
        return
    try:
        import contextlib
        import ctypes
        import types

        mod = types.ModuleType("antenv.axon_hooks")
        _state = {"hook": None}

        def set_axon_ntff_profile_hook(hook):
            _state["hook"] = hook

        def get_axon_ntff_profile_hook():
            if _state["hook"] is None:
                try:
                    lib = ctypes.CDLL("/opt/axon/libaxon_pjrt.so")
                except OSError:
                    return None
                if not hasattr(lib, "axon_start_nrt_profile"):
                    return None
                lib.axon_start_nrt_profile.argtypes = [ctypes.POINTER(ctypes.c_int64), ctypes.c_size_t]
                lib.axon_start_nrt_profile.restype = ctypes.c_int64
                lib.axon_stop_nrt_profile.argtypes = [ctypes.c_char_p]
                lib.axon_stop_nrt_profile.restype = ctypes.c_int64

                @contextlib.contextmanager
                def _hook(output_dir, device_ids):
                    import jax

                    jax.devices()
                    if device_ids:
                        ids = (ctypes.c_int64 * len(device_ids))(*device_ids)
                        rc = lib.axon_start_nrt_profile(ids, len(device_ids))
                    else:
                        rc = lib.axon_start_nrt_profile(None, 0)
                    if rc != 0:
                        raise RuntimeError(f"axon_start_nrt_profile rc={rc}")
                    try:
                        yield
                    finally:
                        n = lib.axon_stop_nrt_profile(str(output_dir).encode())
                        print(f"profile: {n} file(s) written to {output_dir}", file=sys.stderr)

                _state["hook"] = _hook
            return _state["hook"]

        mod.set_axon_ntff_profile_hook = set_axon_ntff_profile_hook
        mod.get_axon_ntff_profile_hook = get_axon_ntff_profile_hook
        sys.modules["antenv.axon_hooks"] = mod
        try:
            import antenv

            antenv.axon_hooks = mod
        except ImportError:
            pass
    except Exception:
        pass


def run(inputs, trace=False, **kw):
    _ensure_ntff_hook_shim()
    from concourse import bass_utils as BU
    from concourse.bass_utils import run_bass_kernel_spmd

    if not getattr(BU.upload_artifacts, "_safe", False):
        _orig_upload = BU.upload_artifacts

        def _safe_upload(tmpdir):
            try:
                return _orig_upload(tmpdir)
            except Exception:
                return f"local:{tmpdir}"

        _safe_upload._safe = True
        BU.upload_artifacts = _safe_upload

    nc = _get_nc()
    in_maps = make_in_maps(**inputs)
    res = run_bass_kernel_spmd(nc, in_maps, list(range(H)), trace=trace, **kw)
    bo = np.asarray(inputs["bo"], np.float32)
    out = np.zeros((B, D), np.float32)
    for c in range(H):
        out += res.results[c]["y"]
    out += bo[None, :]
    return out, res


def kernel(**inputs):
    out, _ = run(inputs, trace=False)
    return out


# revision 48
# speedup vs baseline: 1.0237x; 1.0237x over previous
"""Multi-head cross-batch attention (B=4096, d_model=512, H=8 heads) on 8 TRN2 cores.

Sharding: one head per NeuronCore (tensor-parallel over H). Each core computes
its head's Q/K/V projections from a replicated (pre-transposed) x, the full
[4096, 4096] score block for that head, softmax (transposed layout, denominator
via a ones-column in V), attn @ V, and its partial out-projection
Y_h = attn_h @ Wo[:, h*64:(h+1)*64].T. Host sums the 8 partials and adds bo.

Layout notes (per core):
  - xT [512, 4096] (c on partitions) is fed from host so every matmul can
    contract over the partition dim without any on-device transpose of x.
  - QT/KT are stored duplicated across partition halves ([128, 4096]) so score
    matmuls can be row-packed two-at-a-time into the 128x128 PE array (the
    contraction dim is only 64).
  - Scores are computed transposed (ST[j, m]) so softmax's sum over keys j can
    ride the attn@V matmul: V is augmented with a ones column, making the
    accumulated output row 64 equal to sum_j exp(s). No max-subtraction is
    needed: scores are O(1) here (verified), so exp cannot overflow.
  - Normalization commutes with the out-projection, so Y_un rows are scaled by
    1/r with a per-partition tensor_scalar after the final matmul. r (living in
    a free-dim row) is transposed to partitions with a K=1 matmul.
  - Matmul inputs are bf16 (1 PE pass vs 2 for fp32, fast weight load);
    accumulation is fp32 in PSUM, exp inputs and the softmax denominator stay
    fp32. Emulated error of this variant vs the f64 reference: l2 rel 1.9e-3.
  - Emission is software-pipelined: attnV trails its scores/exp by one group so
    the PE's in-order queue never blocks on an in-flight exp; each chunk's
    output phase is emitted mid-way through the next chunk; chunk 0's score
    groups are interleaved into the projection loop as x arrives n-major.
"""

import sys

if "/opt/trn_rl_repo" not in sys.path:
    sys.path.insert(0, "/opt/trn_rl_repo")

import ml_dtypes
import numpy as np

import concourse.bass as bass
import concourse.tile as tile
from concourse import bacc, mybir

B = 4096
D = 512
H = 8
DK = 64
MC = 512  # query-chunk (m) width
N_MC = B // MC  # 8
JB = B // 128  # 32 j-blocks of 128 keys
F32 = mybir.dt.float32
BF16 = mybir.dt.bfloat16
MM_DT = BF16
NP_MM_DT = ml_dtypes.bfloat16 if MM_DT == BF16 else np.float32

# j-blocks per score/exp group: 3 blocks = 1536 floats = 3 PSUM banks.
# PSUM budget: 2x3 (score staging) + 1 (attnV accum) + 1 (outproj/rT/Vproj) = 8.
JGROUPS = [(0, 3), (3, 3), (6, 3), (9, 3), (12, 3), (15, 3), (18, 3), (21, 3), (24, 3), (27, 3), (30, 2)]

_NC_CACHE = None


def build_nc():
    nc = bacc.Bacc()

    xt = nc.dram_tensor("xt", [D, B], MM_DT, kind="ExternalInput")
    wqt = nc.dram_tensor("wqt", [D, 128], MM_DT, kind="ExternalInput")  # [c, d dup'd]
    wkt = nc.dram_tensor("wkt", [D, 128], MM_DT, kind="ExternalInput")
    wvt = nc.dram_tensor("wvt", [D, DK], MM_DT, kind="ExternalInput")  # [c, d]
    bqd = nc.dram_tensor("bqd", [128, 1], F32, kind="ExternalInput")  # bias dup'd
    bkd = nc.dram_tensor("bkd", [128, 1], F32, kind="ExternalInput")
    bvr = nc.dram_tensor("bvr", [1, DK], MM_DT, kind="ExternalInput")  # bias as row
    wot = nc.dram_tensor("wot", [DK, D], MM_DT, kind="ExternalInput")
    y = nc.dram_tensor("y", [B, D], F32, kind="ExternalOutput")

    with tile.TileContext(nc) as tc:
        with (
            tc.tile_pool(name="const", bufs=1) as const,
            tc.tile_pool(name="epool", bufs=20) as epool,
            tc.tile_pool(name="otpool", bufs=2) as otpool,
            tc.tile_pool(name="ypool", bufs=3) as ypool,
            tc.tile_pool(name="rpool", bufs=4) as rpool,
            tc.tile_pool(name="score_ps", bufs=2, space="PSUM") as score_ps,
            tc.tile_pool(name="attnv_ps", bufs=1, space="PSUM") as attnv_ps,
            tc.tile_pool(name="out_ps", bufs=1, space="PSUM") as out_ps,
        ):
            # ---- persistent SBUF ----
            x_sb = const.tile([128, 4 * B], MM_DT)  # 4 c-chunks side by side
            wq_sb = const.tile([128, 512], MM_DT)  # 4 c-chunks of [128,128]
            wk_sb = const.tile([128, 512], MM_DT)
            wv_sb = const.tile([128, 4 * DK], MM_DT)  # 4 c-chunks of [128,64]
            bq_sb = const.tile([128, 1], F32)
            bk_sb = const.tile([128, 1], F32)
            bv_sb = const.tile([1, DK], MM_DT)
            wot_sb = const.tile([DK, D], MM_DT)
            ones_sb = const.tile([128, 1], F32)
            onesr_sb = const.tile([1, 128], MM_DT)
            qt_sb = const.tile([128, B], MM_DT)  # QT dup'd across partition halves
            kt_sb = const.tile([128, B], MM_DT)
            vp_sb = const.tile([128, JB * (DK + 1)], MM_DT)  # [V | 1] per j-block

            # ---- input DMAs ----
            # Weights first (tiny, gate every projection matmul), one descriptor
            # per tensor via 3D APs. x streams n-major as 8 consolidated DMAs
            # split across two otherwise-idle engines so projections/scores/exp
            # start after ~1/8 of the x DMA.
            nc.sync.dma_start(
                out=wq_sb[:].rearrange("p (c n) -> p c n", c=4),
                in_=wqt[:].rearrange("(c p) n -> p c n", p=128),
            )
            nc.sync.dma_start(
                out=wk_sb[:].rearrange("p (c n) -> p c n", c=4),
                in_=wkt[:].rearrange("(c p) n -> p c n", p=128),
            )
            nc.sync.dma_start(
                out=wv_sb[:].rearrange("p (c n) -> p c n", c=4),
                in_=wvt[:].rearrange("(c p) n -> p c n", p=128),
            )
            nc.sync.dma_start(out=bq_sb[:], in_=bqd[:])
            nc.sync.dma_start(out=bk_sb[:], in_=bkd[:])
            nc.sync.dma_start(out=bv_sb[:], in_=bvr[:])
            nc.sync.dma_start(out=wot_sb[:], in_=wot[:])
            x_sb3 = x_sb[:].rearrange("p (c n) -> p c n", c=4)
            xt3 = xt[:].rearrange("(c p) n -> p c n", p=128)
            x_dma_eng = [nc.gpsimd, nc.scalar, nc.sync]
            for n in range(N_MC):
                x_dma_eng[n % 3].dma_start(
                    out=x_sb3[:, :, n * MC : (n + 1) * MC],
                    in_=xt3[:, :, n * MC : (n + 1) * MC],
                )
            nc.vector.memset(ones_sb[:], 1.0)
            nc.vector.memset(onesr_sb[:], 1.0)
            nc.vector.memset(vp_sb[:], 1.0)  # ones columns; V data overwrites the rest

            # ---- emission helpers ----
            def emit_qk(n):
                # one 2-bank PSUM unit holds Q (cols 0:512) and K (cols 512:1024)
                pp = score_ps.tile([128, 2 * MC], F32, tag="score")
                for off, w_sb in ((0, wq_sb), (MC, wk_sb)):
                    for c in range(4):
                        nc.tensor.matmul(
                            pp[:, off : off + MC],
                            w_sb[:, c * 128 : (c + 1) * 128],
                            x_sb[:, c * B + n * MC : c * B + (n + 1) * MC],
                            start=(c == 0),
                            stop=(c == 3),
                        )
                for off, b_sb, dst in ((0, bq_sb, qt_sb), (MC, bk_sb, kt_sb)):
                    nc.vector.tensor_scalar(
                        out=dst[:, n * MC : (n + 1) * MC], in0=pp[:, off : off + MC],
                        scalar1=b_sb[:], scalar2=None, op0=mybir.AluOpType.add,
                    )

            def emit_v4(n):
                # V for j-blocks 4n..4n+3 in one 1-bank PSUM unit, one strided drain
                t0 = 4 * n
                vps = out_ps.tile([128, 4 * DK], F32, tag="out")
                for k in range(4):
                    dst = vps[:, k * DK : (k + 1) * DK]
                    for c in range(4):
                        nc.tensor.matmul(
                            dst,
                            x_sb[:, c * B + (t0 + k) * 128 : c * B + (t0 + k + 1) * 128],
                            wv_sb[:, c * DK : (c + 1) * DK],
                            start=(c == 0),
                            stop=False,
                        )
                    nc.tensor.matmul(dst, onesr_sb[:], bv_sb[:], start=False, stop=True)
                nc.vector.tensor_copy(
                    vp_sb[:].rearrange("p (t e) -> p t e", e=DK + 1)[:, t0 : t0 + 4, 0:DK],
                    vps[:].rearrange("p (t e) -> p t e", e=DK),
                )

            def emit_sc(mc, g0, gn):
                m0 = mc * MC
                sp = score_ps.tile([128, gn * MC], F32, tag="score")
                et = epool.tile([128, gn * MC], MM_DT, tag="E")
                for k in range(gn):
                    jb = g0 + k
                    h0 = 64 * (jb % 2)
                    nc.tensor.matmul(
                        sp[:, k * MC : (k + 1) * MC],
                        kt_sb[h0 : h0 + 64, jb * 128 : (jb + 1) * 128],
                        qt_sb[h0 : h0 + 64, m0 : m0 + MC],
                        start=True,
                        stop=True,
                    )
                nc.scalar.activation(et[:], sp[:], mybir.ActivationFunctionType.Exp, scale=0.125)
                return et

            def emit_av(mc, g0, gn, et, av):
                for k in range(gn):
                    jb = g0 + k
                    nc.tensor.matmul(
                        av[:],
                        vp_sb[:, jb * (DK + 1) : (jb + 1) * (DK + 1)],
                        et[:, k * MC : (k + 1) * MC],
                        start=(jb == 0),
                        stop=(jb == JB - 1),
                    )

            def emit_otcopy(av):
                # r row (f32, feeds the K=1 transpose matmul) + bf16 numerator
                ot_f = otpool.tile([DK + 1, MC], F32, tag="otf")
                nc.vector.tensor_copy(ot_f[DK : DK + 1, :], av[DK : DK + 1, :])
                ot_b = otpool.tile([DK, MC], MM_DT, tag="otb")
                nc.vector.tensor_copy(ot_b[:], av[0:DK, :])
                return ot_f, ot_b

            def emit_output(mc, ot_f, ot_b):
                m0 = mc * MC
                for q in range(4):
                    rt = out_ps.tile([128, MC], F32, tag="out")
                    nc.tensor.matmul(
                        rt[:, 0:1],
                        ot_f[DK : DK + 1, q * 128 : (q + 1) * 128],
                        ones_sb[DK : DK + 1, 0:1],
                        start=True,
                        stop=True,
                    )
                    rv = rpool.tile([128, 1], F32, tag="rinv")
                    nc.vector.reciprocal(rv[:], rt[:, 0:1])
                    yp = out_ps.tile([128, MC], F32, tag="out")
                    nc.tensor.matmul(yp[:], ot_b[:, q * 128 : (q + 1) * 128], wot_sb[:], start=True, stop=True)
                    ysb = ypool.tile([128, MC], F32, tag="y")
                    nc.vector.tensor_scalar(
                        out=ysb[:], in0=yp[:], scalar1=rv[:], scalar2=None, op0=mybir.AluOpType.mult
                    )
                    nc.sync.dma_start(out=y[m0 + q * 128 : m0 + (q + 1) * 128, :], in_=ysb[:])

            # ---- software-pipelined main emission ----
            # attnV for a group is emitted one group behind its scores/exp, so
            # the PE's in-order queue never blocks on an in-flight exp. Each
            # chunk's output phase is emitted mid-way through the next chunk.
            state = {"prev": None, "av": None, "out_pending": None}

            def drain_prev():
                if state["prev"] is None:
                    return
                mc, g0, gn, et = state["prev"]
                state["prev"] = None
                if state["av"] is None:
                    state["av"] = attnv_ps.tile([DK + 1, MC], F32, tag="attnv", name="av")
                emit_av(mc, g0, gn, et, state["av"])
                if g0 + gn == JB:  # chunk complete
                    ot_f, ot_b = emit_otcopy(state["av"])
                    state["av"] = None
                    if state["out_pending"] is not None:
                        emit_output(*state["out_pending"])
                    state["out_pending"] = (mc, ot_f, ot_b)
                elif g0 >= 12 and state["out_pending"] is not None and state["out_pending"][0] == mc - 1:
                    emit_output(*state["out_pending"])
                    state["out_pending"] = None

            def push(mc, g0, gn):
                et = emit_sc(mc, g0, gn)
                drain_prev()
                state["prev"] = (mc, g0, gn, et)

            # projections interleaved with chunk 0 (x arrives n-major)
            gi = 0
            for n in range(N_MC):
                emit_qk(n)
                emit_v4(n)
                # chunk-0 groups whose j-blocks (and V blocks) are now projected
                while gi < len(JGROUPS) and JGROUPS[gi][0] + JGROUPS[gi][1] <= 4 * n + 4:
                    push(0, *JGROUPS[gi])
                    gi += 1
            while gi < len(JGROUPS):
                push(0, *JGROUPS[gi])
                gi += 1
            for mc in range(1, N_MC):
                for g0, gn in JGROUPS:
                    push(mc, g0, gn)
            drain_prev()
            emit_output(*state["out_pending"])
    nc.finalize()
    return nc


def _get_nc():
    global _NC_CACHE
    if _NC_CACHE is None:
        _NC_CACHE = build_nc()
    return _NC_CACHE


def make_in_maps(x, Wq, bq, Wk, bk, Wv, bv, Wo, bo):
    xT = np.ascontiguousarray(np.asarray(x, dtype=np.float32).T).astype(NP_MM_DT)
    maps = []
    for h in range(H):
        s = slice(h * DK, (h + 1) * DK)
        wqT = np.asarray(Wq, np.float32)[s, :].T  # [512, 64]
        wkT = np.asarray(Wk, np.float32)[s, :].T
        maps.append(
            {
                "xt": xT,
                "wqt": np.ascontiguousarray(np.concatenate([wqT, wqT], axis=1)).astype(NP_MM_DT),
                "wkt": np.ascontiguousarray(np.concatenate([wkT, wkT], axis=1)).astype(NP_MM_DT),
                "wvt": np.ascontiguousarray(np.asarray(Wv, np.float32)[s, :].T).astype(NP_MM_DT),
                "bqd": np.ascontiguousarray(np.tile(np.asarray(bq, np.float32)[s], 2).reshape(128, 1)),
                "bkd": np.ascontiguousarray(np.tile(np.asarray(bk, np.float32)[s], 2).reshape(128, 1)),
                "bvr": np.ascontiguousarray(np.asarray(bv, np.float32)[s].reshape(1, DK)).astype(NP_MM_DT),
                "wot": np.ascontiguousarray(np.asarray(Wo, np.float32)[:, s].T).astype(NP_MM_DT),
            }
        )
    return maps


def _ensure_ntff_hook_shim():
    # The image's antenv package lacks axon_hooks; bass_utils imports it when
    # tracing is requested (including via the BASS_TRACE env var). Register a
    # ctypes-backed shim so that path works regardless of environment.
    if "antenv.axon_hooks" in sys.modules:
        return
    try:
        import contextlib
        import ctypes
        import types

        mod = types.ModuleType("antenv.axon_hooks")
        _state = {"hook": None}

        def set_axon_ntff_profile_hook(hook):
            _state["hook"] = hook

        def get_axon_ntff_profile_hook():
            if _state["hook"] is None:
                try:
                    lib = ctypes.CDLL("/opt/axon/libaxon_pjrt.so")
                except OSError:
                    return None
                if not hasattr(lib, "axon_start_nrt_profile"):
                    return None
                lib.axon_start_nrt_profile.argtypes = [ctypes.POINTER(ctypes.c_int64), ctypes.c_size_t]
                lib.axon_start_nrt_profile.restype = ctypes.c_int64
                lib.axon_stop_nrt_profile.argtypes = [ctypes.c_char_p]
                lib.axon_stop_nrt_profile.restype = ctypes.c_int64

                @contextlib.contextmanager
                def _hook(output_dir, device_ids):
                    import jax

                    jax.devices()
                    if device_ids:
                        ids = (ctypes.c_int64 * len(device_ids))(*device_ids)
                        rc = lib.axon_start_nrt_profile(ids, len(device_ids))
                    else:
                        rc = lib.axon_start_nrt_profile(None, 0)
                    if rc != 0:
                        raise RuntimeError(f"axon_start_nrt_profile rc={rc}")
                    try:
                        yield
                    finally:
                        n = lib.axon_stop_nrt_profile(str(output_dir).encode())
                        print(f"profile: {n} file(s) written to {output_dir}", file=sys.stderr)

                _state["hook"] = _hook
            return _state["hook"]

        mod.set_axon_ntff_profile_hook = set_axon_ntff_profile_hook
        mod.get_axon_ntff_profile_hook = get_axon_ntff_profile_hook
        sys.modules["antenv.axon_hooks"] = mod
        try:
            import antenv

            antenv.axon_hooks = mod
        except ImportError:
            pass
    except Exception:
        pass


def run(inputs, trace=False, **kw):
    _ensure_ntff_hook_shim()
    from concourse import bass_utils as BU
    from concourse.bass_utils import run_bass_kernel_spmd

    if not getattr(BU.upload_artifacts, "_safe", False):
        _orig_upload = BU.upload_artifacts

        def _safe_upload(tmpdir):
            try:
                return _orig_upload(tmpdir)
            except Exception:
                return f"local:{tmpdir}"

        _safe_upload._safe = True
        BU.upload_artifacts = _safe_upload

    nc = _get_nc()
    in_maps = make_in_maps(**inputs)
    res = run_bass_kernel_spmd(nc, in_maps, list(range(H)), trace=trace, **kw)
    bo = np.asarray(inputs["bo"], np.float32)
    out = np.zeros((B, D), np.float32)
    for c in range(H):
        out += res.results[c]["y"]
    out += bo[None, :]
    return out, res


def kernel(**inputs):
    out, _ = run(inputs, trace=False)
    return out
